# revision 2
# baseline (speedup 1.0000x reference)
"""Trainium2 Bass kernel for CascadeClassifierGNN (3-layer GCN + BN + ReLU,
global mean pool, 2-layer MLP head), sharded across 8 NeuronCores.

Push-mode sharding: nodes are partitioned contiguously across the 8 cores;
each core owns the edges whose SOURCE lies in its shard. Per layer, each
core computes u = dinv * (h @ W') for its local nodes (table in local HBM),
then for each destination-owner d: dma_gather batches per-edge source rows
u[src_local] from the local table into SBUF and dma_scatter_add pushes them
into a per-owner partial accumulator in HBM (dst_local indices). Scatter
instructions are scheduled in "rounds" so no destination row repeats within
one instruction (the SDMA CCE read-modify-write races on duplicates).
A ReduceScatter sums the 8 cores' partials and hands each core the
aggregate for its own nodes. BatchNorm is folded into the layer weights on
the host. The global mean pool uses one-hot selection matmuls accumulated
in PSUM followed by a tiny AllReduce; the MLP head runs replicated.
"""

import math
import os

import numpy as np

import concourse.bacc as bacc
import concourse.mybir as mybir
import concourse.tile as tile
from concourse import bass_utils
from concourse import library_config
from concourse.masks import make_identity

F32 = mybir.dt.float32
I32 = mybir.dt.int32
I16 = mybir.dt.int16
ALU = mybir.AluOpType

# Problem configuration (hardcoded per contest contract).
CFG = dict(N=100000, E=1600000, F_IN=10, H=64, B=128, C=3, EPS=1e-5)
M = 8           # cores
P = 128         # partitions
SUPER = 4       # dst tiles per super-tile (dense-pipeline granularity)
CH = 4096       # max indices per gather/scatter instruction (ring fit)

TRACE = os.environ.get("GNN_TRACE", "0") == "1"
LAST_EXEC_NS = None


def _fold_bn(Wl, bl, gl, bel, ml, vl, eps):
    A = (np.asarray(gl, np.float32)
         / np.sqrt(np.asarray(vl, np.float32) + np.float32(eps)))
    Wp = (np.asarray(Wl, np.float32) * A[None, :]).astype(np.float32)
    Bv = ((np.asarray(bl, np.float32) - np.asarray(ml, np.float32)) * A
          + np.asarray(bel, np.float32)).astype(np.float32)
    return Wp, Bv


def preprocess(x, edge_index, batch,
               W1, b1, g1, be1, m1, v1,
               W2, b2, g2, be2, m2, v2,
               W3, b3, g3, be3, m3, v3,
               fw1, fb1, fw2, fb2, cfg=CFG):
    N, E, F_IN, H, B, C = (cfg["N"], cfg["E"], cfg["F_IN"], cfg["H"],
                           cfg["B"], cfg["C"])
    x = np.asarray(x, dtype=np.float32)
    src = np.asarray(edge_index[0], dtype=np.int64)
    dst = np.asarray(edge_index[1], dtype=np.int64)
    batch = np.asarray(batch, dtype=np.int64)

    assert N % M == 0
    NS = N // M
    T = math.ceil(math.ceil(NS / P) / SUPER) * SUPER
    NSP = T * P
    GSUP = T // SUPER
    DEAD = NSP - 1

    deg = (np.bincount(dst, minlength=N) + 1.0).astype(np.float32)
    dinv = (1.0 / np.sqrt(deg)).astype(np.float32)

    owner_s = src // NS
    owner_d = dst // NS
    src_l = (src - owner_s * NS).astype(np.int64)
    dst_l = (dst - owner_d * NS).astype(np.int64)

    # rank of each edge within its (src-owner, dst-owner, dst) bucket
    key = (owner_s * M + owner_d) * NS + dst_l
    order = np.argsort(key, kind="stable")
    ks = key[order]
    first = np.r_[True, ks[1:] != ks[:-1]]
    start_of_grp = np.where(first, np.arange(E), 0)
    start_of_grp = np.maximum.accumulate(start_of_grp)
    rank = np.arange(E) - start_of_grp          # per (c,d,dst) occurrence id
    cd = (ks // NS).astype(np.int64)            # c*M + d per sorted edge
    Rmax = int(rank.max()) + 1

    cnt = np.zeros((M * M, Rmax), np.int64)
    np.add.at(cnt, (cd, rank), 1)
    cnt = cnt.reshape(M, M, Rmax)
    cnt_dr = cnt.max(axis=0)                    # [d, r] max over src cores
    cnt_dr_p = ((cnt_dr + P - 1) // P) * P      # pad rounds to 128-multiples
    cnt_dr_p[cnt_dr == 0] = 0

    # chunk schedule, identical on every core: d-major, round-major
    sched = []            # (d, start_in_stream, length)
    off_dr = np.zeros((M, Rmax), np.int64)
    d_base = np.zeros(M + 1, np.int64)
    pos = 0
    for d in range(M):
        d_base[d] = pos
        for r in range(Rmax):
            ln = int(cnt_dr_p[d, r])
            if ln == 0:
                continue
            off_dr[d, r] = pos
            o = 0
            while o < ln:
                piece = min(CH, ln - o)
                sched.append((d, pos + o - d_base[d], piece))
                o += piece
            pos += ln
    d_base[M] = pos
    SUM = pos
    SUMC = SUM // 16

    # per-core edge placement into the stream
    # order2: edges sorted by (c, d, r) with dst ascending inside each round
    key2 = (cd * Rmax + rank)
    order2 = np.argsort(key2, kind="stable")    # stable keeps dst-ascending
    k2 = key2[order2]
    first2 = np.r_[True, k2[1:] != k2[:-1]]
    start2 = np.where(first2, np.arange(E), 0)
    start2 = np.maximum.accumulate(start2)
    pos_in_round = np.arange(E) - start2

    e_c = (cd[order2] // M)
    e_d = (cd[order2] % M)
    e_r = rank[order2]
    e_src = src_l[order][order2]
    e_dst = dst_l[order][order2]
    stream_pos = d_base[e_d] + off_dr[e_d, e_r] - d_base[e_d] \
        + off_dr[e_d, e_r] * 0 + pos_in_round
    # off_dr is absolute; recompute relative-to-core layout:
    stream_pos = off_dr[e_d, e_r] + pos_in_round

    gstream = np.full((M, SUM), DEAD, np.int16)
    sstream = np.full((M, SUM), DEAD, np.int16)
    gstream[e_c, stream_pos] = e_src.astype(np.int16)
    sstream[e_c, stream_pos] = e_dst.astype(np.int16)

    # wrap to the Q7 16-partition layout: stream pos p -> [p%16, p//16]
    g16 = gstream.reshape(M, SUMC, 16).transpose(0, 2, 1).copy()
    s16 = sstream.reshape(M, SUMC, 16).transpose(0, 2, 1).copy()

    # node-side shards
    xs = np.zeros((M, NSP, F_IN), dtype=np.float32)
    dinv_t = np.zeros((M, P, T), dtype=np.float32)
    batch_sh = np.full((M, NSP), -1, dtype=np.int32)
    for c in range(M):
        lo = c * NS
        xs[c, :NS] = x[lo:lo + NS]
        dv = np.zeros(NSP, np.float32)
        dv[:NS] = dinv[lo:lo + NS]
        dinv_t[c] = dv.reshape(T, P).T
        batch_sh[c, :NS] = batch[lo:lo + NS]

    counts = np.bincount(batch, minlength=B).astype(np.float32)
    cinv = (1.0 / np.maximum(counts, 1.0)).astype(np.float32)

    eps = cfg["EPS"]
    W1p, B1 = _fold_bn(W1, b1, g1, be1, m1, v1, eps)
    W2p, B2 = _fold_bn(W2, b2, g2, be2, m2, v2, eps)
    W3p, B3 = _fold_bn(W3, b3, g3, be3, m3, v3, eps)

    def bc(v, reps):
        return np.ascontiguousarray(
            np.tile(np.asarray(v, np.float32)[None, :], (P, reps)))

    iota = np.tile(np.arange(P, dtype=np.float32)[None, :], (P, 1))

    shared = {
        "W1p": W1p, "W2p": W2p, "W3p": W3p,
        "B1bc": bc(B1, SUPER), "B2bc": bc(B2, SUPER), "B3bc": bc(B3, SUPER),
        "fw1": np.asarray(fw1, np.float32), "fw2": np.asarray(fw2, np.float32),
        "fb1bc": bc(fb1, 1), "fb2bc": bc(fb2, 1),
        "cinv": cinv.reshape(B, 1), "iota": iota,
    }

    in_maps = []
    for c in range(M):
        im = {
            "x_sh": xs[c],
            "dinv_t": dinv_t[c],
            "batch_sh": batch_sh[c].reshape(NSP, 1),
            "g16": g16[c],
            "s16": s16[c],
        }
        im.update(shared)
        in_maps.append(im)

    meta = dict(cfg=tuple(sorted(cfg.items())), NS=NS, NSP=NSP, T=T,
                GSUP=GSUP, SUM=SUM, SUMC=SUMC,
                sched=tuple(sched), d_cols=tuple(
                    int(d_base[d + 1] - d_base[d]) // 16 for d in range(M)),
                d_base=tuple(int(v) for v in d_base))
    return in_maps, meta


def cache_key(meta):
    return (meta["cfg"], meta["sched"])


def build_program(meta):
    cfg = dict(meta["cfg"])
    F_IN, H, B, C = cfg["F_IN"], cfg["H"], cfg["B"], cfg["C"]
    NSP, T, GSUP = meta["NSP"], meta["T"], meta["GSUP"]
    SUM, SUMC = meta["SUM"], meta["SUMC"]
    sched = meta["sched"]
    d_base = meta["d_base"]
    d_cols = meta["d_cols"]
    SW = SUPER * H
    HB = H // 2

    nc = bacc.Bacc("TRN2", target_bir_lowering=False, debug=False,
                   num_devices=M)

    x_sh = nc.dram_tensor("x_sh", [NSP, F_IN], F32, kind="ExternalInput")
    dinv_t_d = nc.dram_tensor("dinv_t", [P, T], F32, kind="ExternalInput")
    batch_sh = nc.dram_tensor("batch_sh", [NSP, 1], I32, kind="ExternalInput")
    g16_d = nc.dram_tensor("g16", [16, SUMC], I16, kind="ExternalInput")
    s16_d = nc.dram_tensor("s16", [16, SUMC], I16, kind="ExternalInput")
    W1p = nc.dram_tensor("W1p", [F_IN, H], F32, kind="ExternalInput")
    W2p = nc.dram_tensor("W2p", [H, H], F32, kind="ExternalInput")
    W3p = nc.dram_tensor("W3p", [H, H], F32, kind="ExternalInput")
    Bbc_d = [nc.dram_tensor(f"B{l}bc", [P, SW], F32, kind="ExternalInput")
             for l in (1, 2, 3)]
    fw1_d = nc.dram_tensor("fw1", [H, HB], F32, kind="ExternalInput")
    fw2_d = nc.dram_tensor("fw2", [HB, C], F32, kind="ExternalInput")
    fb1bc = nc.dram_tensor("fb1bc", [P, HB], F32, kind="ExternalInput")
    fb2bc = nc.dram_tensor("fb2bc", [P, C], F32, kind="ExternalInput")
    cinv_d = nc.dram_tensor("cinv", [B, 1], F32, kind="ExternalInput")
    iota_d = nc.dram_tensor("iota", [P, P], F32, kind="ExternalInput")
    out_d = nc.dram_tensor("out", [B, C], F32, kind="ExternalOutput")

    table = nc.dram_tensor("table", [NSP, H], F32)
    partials = nc.dram_tensor("partials", [M * NSP, H], F32)
    agg = nc.dram_tensor("agg", [NSP, H], F32)
    gidxrep = nc.dram_tensor("gidxrep", [P, SUMC], I16)
    sidxrep = nc.dram_tensor("sidxrep", [P, SUMC], I16)
    pool_in = nc.dram_tensor("pool_in", [B, H], F32)
    pool_out = nc.dram_tensor("pool_out", [B, H], F32)

    groups = [list(range(M))]
    maxcols = max(d_cols)

    def super_rows(dram, g):
        rows = dram[g * SUPER * P:(g + 1) * SUPER * P, :]
        return rows.rearrange("(t p) j -> p t j", p=P)

    with tile.TileContext(nc) as tc:
        with (
            tc.tile_pool(name="resident", bufs=1) as rp,
            tc.tile_pool(name="work", bufs=2) as wp,
            tc.tile_pool(name="idx", bufs=2) as ip,
            tc.tile_pool(name="gather", bufs=3) as gp,
            tc.tile_pool(name="psum", bufs=2, space="PSUM") as pp,
            tc.tile_pool(name="psum_acc", bufs=1, space="PSUM") as pacc,
        ):
            ident = rp.tile([P, P], F32, tag="ident")
            make_identity(nc, ident[:])
            nc.gpsimd.load_library(library_config.mlp)

            iota_f = rp.tile([P, P], F32, tag="iota_f")
            nc.sync.dma_start(out=iota_f[:], in_=iota_d[:, :])
            ones_t = rp.tile([P, H], F32, tag="ones")
            nc.vector.memset(ones_t[:], 1.0)
            zeros_t = rp.tile([P, T * H], F32, tag="zeros")
            nc.vector.memset(zeros_t[:], 0.0)

            dinv_ts = rp.tile([P, T], F32, tag="dinv_ts")
            nc.sync.dma_start(out=dinv_ts[:], in_=dinv_t_d[:, :])
            dinvbc = rp.tile([P, T * H], F32, tag="dinvbc")
            for t in range(T):
                nc.vector.tensor_scalar(
                    dinvbc[:, t * H:(t + 1) * H], ones_t[:],
                    dinv_ts[:, t:t + 1], None, ALU.mult)

            selfb = rp.tile([P, T * H], F32, tag="selfb")
            r3b = rp.tile([P, T * H], F32, tag="r3b")

            w1s = rp.tile([F_IN, H], F32, tag="w1s")
            nc.sync.dma_start(out=w1s[:], in_=W1p[:, :])
            w2s = rp.tile([H, H], F32, tag="w2s")
            nc.sync.dma_start(out=w2s[:], in_=W2p[:, :])
            w3s = rp.tile([H, H], F32, tag="w3s")
            nc.sync.dma_start(out=w3s[:], in_=W3p[:, :])
            bbc = []
            for l in range(3):
                t_ = rp.tile([P, SW], F32, tag=f"bbc{l}")
                nc.sync.dma_start(out=t_[:], in_=Bbc_d[l][:, :])
                bbc.append(t_)
            fw1s = rp.tile([H, HB], F32, tag="fw1s")
            nc.sync.dma_start(out=fw1s[:], in_=fw1_d[:, :])
            fw2s = rp.tile([HB, C], F32, tag="fw2s")
            nc.sync.dma_start(out=fw2s[:], in_=fw2_d[:, :])
            fb1s = rp.tile([P, HB], F32, tag="fb1s")
            nc.sync.dma_start(out=fb1s[:], in_=fb1bc[:, :])
            fb2s = rp.tile([P, C], F32, tag="fb2s")
            nc.sync.dma_start(out=fb2s[:], in_=fb2bc[:, :])
            cinvs = rp.tile([B, 1], F32, tag="cinvs")
            nc.sync.dma_start(out=cinvs[:], in_=cinv_d[:, :])

            # replicate the 16-row wrapped idx arrays to the 8 Q7-core
            # stripes expected by dma_gather/dma_scatter_add
            bnc = rp.tile([P, SUMC], I16, tag="idx_bounce")
            nc.sync.dma_start(out=bnc[:16, :], in_=g16_d[:, :])
            nc.sync.dma_start(out=bnc[16:32, :], in_=s16_d[:, :])
            for k in range(M):
                nc.sync.dma_start(out=gidxrep[k * 16:(k + 1) * 16, :],
                                  in_=bnc[:16, :])
                nc.sync.dma_start(out=sidxrep[k * 16:(k + 1) * 16, :],
                                  in_=bnc[16:32, :])

            # ---------------- u1 = (dinv * x) @ W1' ----------------
            for g in range(GSUP):
                xt = wp.tile([P, SUPER * F_IN], F32, tag="xt")
                nc.sync.dma_start(
                    out=xt[:].rearrange("p (t j) -> p t j", j=F_IN),
                    in_=super_rows(x_sh, g))
                xd = wp.tile([P, SUPER * F_IN], F32, tag="xd")
                for b_ in range(SUPER):
                    t = g * SUPER + b_
                    nc.vector.tensor_tensor(
                        out=xd[:, b_ * F_IN:(b_ + 1) * F_IN],
                        in0=xt[:, b_ * F_IN:(b_ + 1) * F_IN],
                        in1=dinvbc[:, t * H:t * H + F_IN],
                        op=ALU.mult)
                tr_ps = pp.tile([F_IN, SUPER * P], F32, tag="tr_ps")
                for b_ in range(SUPER):
                    nc.tensor.transpose(
                        out=tr_ps[:, b_ * P:(b_ + 1) * P],
                        in_=xd[:, b_ * F_IN:(b_ + 1) * F_IN],
                        identity=ident[:])
                xdT = wp.tile([F_IN, SUPER * P], F32, tag="xdT")
                nc.vector.tensor_copy(xdT[:], tr_ps[:])
                z_ps = pp.tile([P, SW], F32, tag="z_ps")
                for b_ in range(SUPER):
                    nc.tensor.matmul(
                        out=z_ps[:, b_ * H:(b_ + 1) * H],
                        lhsT=xdT[:, b_ * P:(b_ + 1) * P],
                        rhs=w1s[:], start=True, stop=True)
                nc.vector.tensor_copy(selfb[:, g * SW:(g + 1) * SW], z_ps[:])
                nc.sync.dma_start(
                    out=super_rows(table, g),
                    in_=selfb[:, g * SW:(g + 1) * SW].rearrange(
                        "p (t j) -> p t j", j=H))

            # ---------------- layers ----------------
            for l in range(3):
                # zero the partial accumulators
                for d in range(M):
                    nc.sync.dma_start(
                        out=partials[d * NSP:(d + 1) * NSP, :].rearrange(
                            "(c p) j -> p c j", p=P),
                        in_=zeros_t[:].rearrange("p (c j) -> p c j", j=H))

                # push phase: gather local table rows, scatter-add to
                # per-destination-owner partials
                cur_d = -1
                gtl = stl = None
                for (d, rel_start, ln) in sched:
                    if d != cur_d:
                        cur_d = d
                        c0 = d_base[d] // 16
                        cols = d_cols[d]
                        gtl = ip.tile([P, maxcols], I16, tag="gt")
                        nc.sync.dma_start(out=gtl[:, :cols],
                                          in_=gidxrep[:, c0:c0 + cols])
                        stl = ip.tile([P, maxcols], I16, tag="st")
                        nc.sync.dma_start(out=stl[:, :cols],
                                          in_=sidxrep[:, c0:c0 + cols])
                    buf = gp.tile([P, (CH // P) * H], F32, tag="buf")
                    bufv = buf[:, :(ln // P) * H].rearrange(
                        "p (c j) -> p c j", j=H)
                    i0 = rel_start // 16
                    nc.gpsimd.dma_gather(
                        bufv, table[:, :],
                        gtl[:, i0:i0 + ln // 16], ln, ln, H,
                        single_packet=False)
                    nc.gpsimd.dma_scatter_add(
                        partials[d * NSP:(d + 1) * NSP, :], bufv,
                        stl[:, i0:i0 + ln // 16], ln, ln, H,
                        single_packet=False)

                nc.gpsimd.collective_compute(
                    "ReduceScatter", ALU.add, replica_groups=groups,
                    ins=[partials.ap().opt()], outs=[agg.ap().opt()])

                # rt = relu((agg + u) * dinv + bias); next-layer
                # u' = (dinv * rt) @ W'
                for g in range(GSUP):
                    gslice = slice(g * SW, (g + 1) * SW)
                    aggt = wp.tile([P, SW], F32, tag="aggt")
                    nc.sync.dma_start(
                        out=aggt[:].rearrange("p (t j) -> p t j", j=H),
                        in_=super_rows(agg, g))
                    rt = aggt[:]
                    nc.vector.tensor_add(rt, rt, selfb[:, gslice])
                    nc.vector.tensor_tensor(out=rt, in0=rt,
                                            in1=dinvbc[:, gslice],
                                            op=ALU.mult)
                    nc.vector.tensor_add(rt, rt, bbc[l][:])
                    nc.vector.tensor_scalar_max(rt, rt, 0.0)

                    if l == 2:
                        nc.vector.tensor_copy(r3b[:, gslice], rt)
                        continue

                    vd = wp.tile([P, SW], F32, tag="vd")
                    nc.vector.tensor_tensor(out=vd[:], in0=rt,
                                            in1=dinvbc[:, gslice],
                                            op=ALU.mult)
                    w_s = w2s if l == 0 else w3s
                    tr_ps = pp.tile([H, SUPER * P], F32, tag="tr_ps")
                    for b_ in range(SUPER):
                        nc.tensor.transpose(
                            out=tr_ps[:, b_ * P:(b_ + 1) * P],
                            in_=vd[:, b_ * H:(b_ + 1) * H],
                            identity=ident[:])
                    vdT = wp.tile([H, SUPER * P], F32, tag="vdT")
                    nc.vector.tensor_copy(vdT[:], tr_ps[:])
                    z_ps = pp.tile([P, SW], F32, tag="z_ps")
                    for b_ in range(SUPER):
                        nc.tensor.matmul(
                            out=z_ps[:, b_ * H:(b_ + 1) * H],
                            lhsT=vdT[:, b_ * P:(b_ + 1) * P],
                            rhs=w_s[:], start=True, stop=True)
                    nc.vector.tensor_copy(selfb[:, gslice], z_ps[:])
                    nc.sync.dma_start(
                        out=super_rows(table, g),
                        in_=selfb[:, gslice].rearrange(
                            "p (t j) -> p t j", j=H))

            # ---------------- global mean pool ----------------
            pool_ps = pacc.tile([B, H], F32, tag="pool_ps")
            for t in range(T):
                bt = wp.tile([P, 1], I32, tag="bt")
                nc.sync.dma_start(out=bt[:],
                                  in_=batch_sh[t * P:(t + 1) * P, :])
                btf = wp.tile([P, 1], F32, tag="btf")
                nc.vector.tensor_copy(btf[:], bt[:])
                S = wp.tile([P, P], F32, tag="S")
                nc.vector.tensor_scalar(S[:], iota_f[:], btf[:], None,
                                        ALU.is_equal)
                nc.tensor.matmul(out=pool_ps[:], lhsT=S[:],
                                 rhs=r3b[:, t * H:(t + 1) * H],
                                 start=(t == 0), stop=(t == T - 1))
            pool_sb = wp.tile([B, H], F32, tag="pool_sb")
            nc.vector.tensor_copy(pool_sb[:], pool_ps[:])
            nc.sync.dma_start(out=pool_in[:, :], in_=pool_sb[:])
            nc.gpsimd.collective_compute(
                "AllReduce", ALU.add, replica_groups=groups,
                ins=[pool_in.ap().opt()], outs=[pool_out.ap().opt()])

            pooled = wp.tile([B, H], F32, tag="pooled")
            nc.sync.dma_start(out=pooled[:], in_=pool_out[:, :])
            nc.vector.tensor_scalar(pooled[:], pooled[:], cinvs[:], None,
                                    ALU.mult)

            # ---------------- MLP head ----------------
            trp = pp.tile([H, B], F32, tag="mlp_ps")
            nc.tensor.transpose(out=trp[:], in_=pooled[:], identity=ident[:])
            pT = wp.tile([H, B], F32, tag="pT")
            nc.vector.tensor_copy(pT[:], trp[:])
            h1ps = pp.tile([B, HB], F32, tag="mlp_ps")
            nc.tensor.matmul(out=h1ps[:], lhsT=pT[:], rhs=fw1s[:],
                             start=True, stop=True)
            h1 = wp.tile([B, HB], F32, tag="h1")
            nc.vector.tensor_add(h1[:], h1ps[:], fb1s[:])
            nc.vector.tensor_scalar_max(h1[:], h1[:], 0.0)
            tr2 = pp.tile([HB, B], F32, tag="mlp_ps")
            nc.tensor.transpose(out=tr2[:], in_=h1[:], identity=ident[:])
            h1T = wp.tile([HB, B], F32, tag="h1T")
            nc.vector.tensor_copy(h1T[:], tr2[:])
            o_ps = pp.tile([B, C], F32, tag="mlp_ps")
            nc.tensor.matmul(out=o_ps[:], lhsT=h1T[:], rhs=fw2s[:],
                             start=True, stop=True)
            o_sb = wp.tile([B, C], F32, tag="o_sb")
            nc.vector.tensor_add(o_sb[:], o_ps[:], fb2s[:])
            nc.sync.dma_start(out=out_d[:, :], in_=o_sb[:])

    nc.compile()
    return nc


_CACHE: dict = {}


def kernel(**inputs) -> np.ndarray:
    global LAST_EXEC_NS
    in_maps, meta = preprocess(**inputs)
    key = cache_key(meta)
    nc = _CACHE.get(key)
    if nc is None:
        nc = build_program(meta)
        _CACHE[key] = nc
    res = bass_utils.run_bass_kernel_spmd(
        nc, in_maps, core_ids=list(range(M)), trace=TRACE)
    LAST_EXEC_NS = res.exec_time_ns
    return np.asarray(res.results[0]["out"])
